# revision 9
# baseline (speedup 1.0000x reference)
"""BART decoder layer on 8 TRN2 NeuronCores.

Sharding: data-parallel over (batch, query-half): core c handles batch c//2,
query rows [half*512, half*512+512). Each core computes the full decoder layer
for its 512 query tokens; self/cross K,V are recomputed per core from the full
batch sequence (no collectives).

On-device layout is "transposed": activations live as [feature, token] so every
matmul contracts along the SBUF partition axis. Weights are pre-transposed on
the host and cast to bf16; accumulation is f32 in PSUM, residuals/LayerNorm are
f32. Softmax skips max-subtraction (scores are O(1) here); row sums come from an
extra ones-column appended to V. LayerNorm partition-axis sums use ones-matmuls
on the TensorEngine.
"""

import sys

sys.path.insert(0, "/opt/trn_rl_repo")

import ml_dtypes
import numpy as np

import concourse.bacc as bacc
import concourse.bass as bass
import concourse.mybir as mybir
import concourse.tile as tile

BF = mybir.dt.bfloat16
F32 = mybir.dt.float32
P = 128
Act = mybir.ActivationFunctionType
Alu = mybir.AluOpType


def default_cfg():
    return dict(B=4, T=1024, S=1024, D=1024, H=16, F=4096, eps=1e-5,
                gelu=Act.Gelu)


def _attention(nc, pa, pools, cfg, kv_dram, L, q_sb, msk_dram, res_sb,
               wq_d, wk_d, wv_d, wo_d, bq_c, bk_c, bo_c, bv_row, h_f32, pg):
    """One multi-head attention block, fully in transposed layout.

    pa: phase-scoped SBUF pool.
    kv_dram: [D, L] bf16 dram AP (source tokens for K/V)
    q_sb:    [P, DC, NQ] bf16 sbuf (source for Q)
    msk_dram:[L, NQ] bf16 dram AP (additive mask, transposed)
    res_sb:  [P, DC, NQ] f32 sbuf (residual)
    wq_d/wk_d/wo_d: [DC, P, DC, P] host-tiled bf16; wv_d: [D, D] bf16
    h_f32:   [P, DC, NQ] f32 sbuf out (attn_out + bias + residual)
    Returns (h_bf_tiles, sq_tiles) lists used by LayerNorm stats.
    """
    D, H, NQ = cfg["D"], cfg["H"], cfg["NQ"]
    HD = D // H
    DC, LC = D // P, L // P
    HPC = P // HD  # heads per 128-row chunk
    KB = min(512, L)  # K-proj column block
    VB = min(512, D)  # V-proj column block
    pw, pps, psa, ppv, psm = (pools[k] for k in
                              ("w", "ps_proj", "ps_score", "ps_pv", "small"))

    kv_sb = pa.tile([P, DC, L], BF, tag="kvsrc")
    nc.sync.dma_start(kv_sb[:], kv_dram.rearrange("(c p) n -> p c n", p=P))
    msk_sb = pa.tile([P, LC, NQ], BF, tag="msk")
    nc.sync.dma_start(msk_sb[:], msk_dram.rearrange("(c p) n -> p c n", p=P))

    # V bias broadcast row -> [P, D]
    vbias = pa.tile([P, D], BF, tag="vbias")
    nc.gpsimd.partition_broadcast(vbias[:], bv_row[:1, :])

    # K^T [d_out, k]  (ksb[p, od, k])
    ksb = pa.tile([P, DC, L], BF, tag="ksb")
    for od in range(DC):
        wk_t = pw.tile([P, DC, P], BF, tag="wod", bufs=3)
        nc.sync.dma_start(wk_t[:], wk_d[od])
        for nb in range(L // KB):
            ps = pps.tile([P, KB], F32, tag="proj")
            for c in range(DC):
                nc.tensor.matmul(ps[:], wk_t[:, c, :],
                                 kv_sb[:, c, nb * KB:(nb + 1) * KB],
                                 start=(c == 0), stop=(c == DC - 1))
            nc.scalar.activation(ksb[:, od, nb * KB:(nb + 1) * KB], ps[:],
                                 Act.Identity, bias=bk_c[:, od:od + 1])

    # V natural layout + ones column: vsb[p, t, h*65:(h+1)*65], col 64 of head = 1
    wv_sb = pw.tile([P, DC, D], BF, tag="wvfull", bufs=1)
    nc.sync.dma_start(wv_sb[:], wv_d.rearrange("(c p) n -> p c n", p=P))
    vsb = pa.tile([P, LC, H * (HD + 1)], BF, tag="vsb")
    vsb_r = vsb[:].rearrange("p t (h w) -> p t h w", w=HD + 1)
    for tt in range(LC):
        nc.vector.memset(vsb_r[:, tt, :, HD:HD + 1], 1.0)
        for nb in range(D // VB):
            ps = pps.tile([P, VB], F32, tag="proj")
            for c in range(DC):
                nc.tensor.matmul(ps[:], kv_sb[:, c, tt * P:(tt + 1) * P],
                                 wv_sb[:, c, nb * VB:(nb + 1) * VB],
                                 start=(c == 0), stop=(c == DC - 1))
            hpb = VB // HD  # heads per block
            nc.vector.tensor_tensor(
                vsb_r[:, tt, nb * hpb:(nb + 1) * hpb, :HD],
                ps[:].rearrange("p (h w) -> p h w", w=HD),
                vbias[:, nb * VB:(nb + 1) * VB].rearrange("p (h w) -> p h w", w=HD),
                Alu.add)

    # Q^T [d_out, q]
    qsb = pa.tile([P, DC, NQ], BF, tag="qsb")
    for od in range(DC):
        wq_t = pw.tile([P, DC, P], BF, tag="wod", bufs=3)
        nc.sync.dma_start(wq_t[:], wq_d[od])
        ps = pps.tile([P, NQ], F32, tag="proj")
        for c in range(DC):
            nc.tensor.matmul(ps[:], wq_t[:, c, :], q_sb[:, c, :],
                             start=(c == 0), stop=(c == DC - 1))
        nc.scalar.activation(qsb[:, od, :], ps[:], Act.Identity,
                             bias=bq_c[:, od:od + 1])

    # attention per head -> osb [d, q] bf16
    osb = pa.tile([P, DC, NQ], BF, tag="osb")
    for h in range(H):
        od, po = divmod(h, HPC)
        po *= HD
        pT = pa.tile([P, LC, NQ], BF, tag="pT", bufs=1)
        for kc in range(LC):
            ps_s = psa.tile([P, NQ], F32, tag="score")
            nc.tensor.matmul(ps_s[:], ksb[po:po + HD, od, kc * P:(kc + 1) * P],
                             qsb[po:po + HD, od, :], start=True, stop=True)
            sc = pa.tile([P, NQ], F32, tag="sc", bufs=2)
            nc.vector.tensor_tensor(sc[:], ps_s[:], msk_sb[:, kc, :], Alu.add)
            nc.scalar.activation(pT[:, kc, :], sc[:], Act.Exp)
        ps_o = ppv.tile([HD + 1, NQ], F32, tag="pv")
        for kc in range(LC):
            nc.tensor.matmul(ps_o[:], vsb[:, kc, h * (HD + 1):(h + 1) * (HD + 1)],
                             pT[:, kc, :], start=(kc == 0), stop=(kc == LC - 1))
        rl = psm.tile([1, NQ], F32, tag="rl", bufs=2)
        nc.vector.reciprocal(rl[:], ps_o[HD:HD + 1, :])
        rb = psm.tile([HD, NQ], F32, tag="rb", bufs=2)
        nc.gpsimd.partition_broadcast(rb[:], rl[:1, :])
        nc.vector.tensor_tensor(osb[po:po + HD, od, :], ps_o[:HD, :], rb[:],
                                Alu.mult)

    # out-proj + bias + residual -> h_f32 (f32); emit bf16 + square tiles for LN
    h_bf_tiles, sq_tiles = [], []
    for od in range(DC):
        wo_t = pw.tile([P, DC, P], BF, tag="wod", bufs=3)
        nc.sync.dma_start(wo_t[:], wo_d[od])
        ps = pps.tile([P, NQ], F32, tag="proj")
        for c in range(DC):
            nc.tensor.matmul(ps[:], wo_t[:, c, :], osb[:, c, :],
                             start=(c == 0), stop=(c == DC - 1))
        nc.vector.scalar_tensor_tensor(h_f32[:, od, :], ps[:], bo_c[:, od:od + 1],
                                       res_sb[:, od, :], Alu.add, Alu.add)
        hb = pg.tile([P, NQ], BF, tag="rot_hbf", bufs=2)
        nc.scalar.copy(hb[:], h_f32[:, od, :])
        sq = pg.tile([P, NQ], BF, tag="rot_sq", bufs=2)
        nc.vector.tensor_tensor(sq[:], hb[:], hb[:], Alu.mult)
        h_bf_tiles.append(hb)
        sq_tiles.append(sq)
    return h_bf_tiles, sq_tiles


def _layernorm(nc, pa, pools, cfg, h_f32, h_bf_tiles, sq_tiles, g_c, b_c,
               y_f32, y_bf):
    """y = LN(h) over the partition (feature) axis via ones-matmul stats."""
    D, NQ, eps = cfg["D"], cfg["NQ"], cfg["eps"]
    DC = D // P
    psm, pstat = pools["small"], pools["ps_stat"]
    ones_bf = pools["ones_bf"]

    ps_sx = pstat.tile([1, NQ], F32, tag="sx")
    for c in range(DC):
        nc.tensor.matmul(ps_sx[:], ones_bf[:], h_bf_tiles[c][:],
                         start=(c == 0), stop=(c == DC - 1))
    ps_sq = pstat.tile([1, NQ], F32, tag="sq")
    for c in range(DC):
        nc.tensor.matmul(ps_sq[:], ones_bf[:], sq_tiles[c][:],
                         start=(c == 0), stop=(c == DC - 1))

    m = psm.tile([1, NQ], F32, tag="m")
    nc.vector.tensor_scalar_mul(m[:], ps_sx[:], 1.0 / D)
    ms = psm.tile([1, NQ], F32, tag="ms")
    nc.vector.tensor_scalar_mul(ms[:], ps_sq[:], 1.0 / D)
    var = psm.tile([1, NQ], F32, tag="var")
    nc.vector.tensor_tensor(var[:], m[:], m[:], Alu.mult)
    nc.vector.tensor_sub(var[:], ms[:], var[:])
    nc.vector.tensor_scalar_add(var[:], var[:], float(eps))
    sd = psm.tile([1, NQ], F32, tag="sd")
    nc.scalar.activation(sd[:], var[:], Act.Sqrt)
    rstd = psm.tile([1, NQ], F32, tag="rstd")
    nc.vector.reciprocal(rstd[:], sd[:])
    cc = psm.tile([1, NQ], F32, tag="cc")
    nc.vector.tensor_tensor(cc[:], m[:], rstd[:], Alu.mult)
    nc.vector.tensor_scalar_mul(cc[:], cc[:], -1.0)

    ab = psm.tile([P, NQ], F32, tag="ab")
    nc.gpsimd.partition_broadcast(ab[:], rstd[:1, :])
    cb = psm.tile([P, NQ], F32, tag="cb")
    nc.gpsimd.partition_broadcast(cb[:], cc[:1, :])

    for c in range(DC):
        t1 = pa.tile([P, NQ], F32, tag="sc", bufs=2)
        nc.vector.tensor_tensor(t1[:], h_f32[:, c, :], ab[:], Alu.mult)
        nc.vector.tensor_tensor(t1[:], t1[:], cb[:], Alu.add)
        nc.vector.tensor_scalar(y_f32[:, c, :], t1[:], g_c[:, c:c + 1],
                                b_c[:, c:c + 1], Alu.mult, Alu.add)
        if y_bf is not None:
            nc.scalar.copy(y_bf[:, c, :], y_f32[:, c, :])


def build_nc(cfg):
    B, T, S, D, H, F = (cfg[k] for k in "BTSDHF")
    NQ = cfg["NQ"] = T // 2
    DC, TC, SC, FC = D // P, T // P, S // P, F // P
    HD = D // H

    nc = bacc.Bacc("TRN2", target_bir_lowering=False,
                   debug=cfg.get("debug", False), num_devices=2 * B)
    dp = nc.declare_dram_parameter
    xT_d = dp("xT", [D, T], BF, isOutput=False)
    xqT_d = dp("xqT", [D, NQ], BF, isOutput=False)
    xres_d = dp("xres", [D, NQ], F32, isOutput=False)
    encT_d = dp("encT", [D, S], BF, isOutput=False)
    mskT_d = dp("mskT", [T, NQ], BF, isOutput=False)
    emskT_d = dp("emskT", [S, NQ], BF, isOutput=False)
    w_d = {}
    for nm in ("sa_wq", "sa_wk", "sa_wo", "ca_wq", "ca_wk", "ca_wo"):
        w_d[nm] = dp(nm + "T", [DC, P, DC, P], BF, isOutput=False)
    for nm in ("sa_wv", "ca_wv"):
        w_d[nm] = dp(nm + "T", [D, D], BF, isOutput=False)
    f1_d = dp("f1T", [FC, P, DC, P], BF, isOutput=False)
    f2_d = dp("f2T", [DC, P, FC, P], BF, isOutput=False)
    bias_cols = {}
    for nm in ("sa_bq", "sa_bk", "sa_bo", "ca_bq", "ca_bk", "ca_bo",
               "fc2_b", "ln1_g", "ln1_b", "ln2_g", "ln2_b", "ln3_g", "ln3_b"):
        bias_cols[nm] = dp(nm + "_c", [P, DC], F32, isOutput=False)
    bias_cols["fc1_b"] = dp("fc1_b_c", [P, FC], F32, isOutput=False)
    bvrow_d = dp("sa_bv_row", [1, D], BF, isOutput=False)
    cvrow_d = dp("ca_bv_row", [1, D], BF, isOutput=False)
    outT_d = dp("outT", [D, NQ], F32, isOutput=True)

    with tile.TileContext(nc) as tc:
        with tc.tile_pool(name="const", bufs=1) as pc, \
             tc.tile_pool(name="glob", bufs=1) as pg, \
             tc.tile_pool(name="wpool", bufs=1) as pw, \
             tc.tile_pool(name="small", bufs=1) as psm, \
             tc.tile_pool(name="ps_proj", bufs=2, space="PSUM") as pps, \
             tc.tile_pool(name="ps_score", bufs=2, space="PSUM") as psa, \
             tc.tile_pool(name="ps_pv", bufs=2, space="PSUM") as ppv, \
             tc.tile_pool(name="ps_stat", bufs=1, space="PSUM") as pstat:

            # constants
            ones_bf = pc.tile([P, 1], BF)
            nc.vector.memset(ones_bf[:], 1.0)
            bc_sb = {}
            for nm, d in bias_cols.items():
                t = pc.tile(list(d.shape), F32, tag="bc_" + nm)
                nc.sync.dma_start(t[:], d[:])
                bc_sb[nm] = t
            bvrow_sb = pc.tile([1, D], BF, tag="bvrow_sa")
            nc.sync.dma_start(bvrow_sb[:], bvrow_d[:])
            cvrow_sb = pc.tile([1, D], BF, tag="bvrow_ca")
            nc.sync.dma_start(cvrow_sb[:], cvrow_d[:])

            pools = dict(w=pw, small=psm, ps_proj=pps, ps_score=psa,
                         ps_pv=ppv, ps_stat=pstat, ones_bf=ones_bf)

            # globals: residual-chain f32 slots and q-source bf16 slots
            xq_sb = pg.tile([P, DC, NQ], BF, tag="qsrc", bufs=2)
            nc.sync.dma_start(xq_sb[:], xqT_d.rearrange("(c p) n -> p c n", p=P))
            xres_sb = pg.tile([P, DC, NQ], F32, tag="af32", bufs=2)
            nc.sync.dma_start(xres_sb[:], xres_d.rearrange("(c p) n -> p c n", p=P))

            # ---- self attention + LN1 ----
            h1 = pg.tile([P, DC, NQ], F32, tag="af32", bufs=2)
            with tc.tile_pool(name="attn1", bufs=1) as pa:
                hbf, sq = _attention(nc, pa, pools, cfg, xT_d, T, xq_sb, mskT_d,
                                     xres_sb, w_d["sa_wq"], w_d["sa_wk"],
                                     w_d["sa_wv"], w_d["sa_wo"],
                                     bc_sb["sa_bq"], bc_sb["sa_bk"], bc_sb["sa_bo"],
                                     bvrow_sb, h1, pg)
                y1 = pg.tile([P, DC, NQ], F32, tag="af32", bufs=2)
                y1b = pg.tile([P, DC, NQ], BF, tag="qsrc", bufs=2)
                _layernorm(nc, pa, pools, cfg, h1, hbf, sq,
                           bc_sb["ln1_g"], bc_sb["ln1_b"], y1, y1b)

            # ---- cross attention + LN2 ----
            h2 = pg.tile([P, DC, NQ], F32, tag="af32", bufs=2)
            with tc.tile_pool(name="attn2", bufs=1) as pa:
                hbf, sq = _attention(nc, pa, pools, cfg, encT_d, S, y1b, emskT_d,
                                     y1, w_d["ca_wq"], w_d["ca_wk"],
                                     w_d["ca_wv"], w_d["ca_wo"],
                                     bc_sb["ca_bq"], bc_sb["ca_bk"], bc_sb["ca_bo"],
                                     cvrow_sb, h2, pg)
                y2 = pg.tile([P, DC, NQ], F32, tag="af32", bufs=2)
                y2b = pg.tile([P, DC, NQ], BF, tag="qsrc", bufs=2)
                _layernorm(nc, pa, pools, cfg, h2, hbf, sq,
                           bc_sb["ln2_g"], bc_sb["ln2_b"], y2, y2b)

            # ---- FFN + LN3 ----
            with tc.tile_pool(name="ffn", bufs=1) as pa:
                fsb = pa.tile([P, FC, NQ], BF, tag="fsb")
                for ft in range(FC):
                    w1 = pw.tile([P, DC, P], BF, tag="wod", bufs=3)
                    nc.sync.dma_start(w1[:], f1_d[ft])
                    ps = pps.tile([P, NQ], F32, tag="proj")
                    for c in range(DC):
                        nc.tensor.matmul(ps[:], w1[:, c, :], y2b[:, c, :],
                                         start=(c == 0), stop=(c == DC - 1))
                    nc.scalar.activation(fsb[:, ft, :], ps[:], cfg["gelu"],
                                         bias=bc_sb["fc1_b"][:, ft:ft + 1])
                h3 = pg.tile([P, DC, NQ], F32, tag="af32", bufs=2)
                hbf, sq = [], []
                for od in range(DC):
                    w2 = pw.tile([P, FC, P], BF, tag="w2", bufs=2)
                    nc.sync.dma_start(w2[:], f2_d[od])
                    ps = pps.tile([P, NQ], F32, tag="proj")
                    for fc_ in range(FC):
                        nc.tensor.matmul(ps[:], w2[:, fc_, :], fsb[:, fc_, :],
                                         start=(fc_ == 0), stop=(fc_ == FC - 1))
                    nc.vector.scalar_tensor_tensor(h3[:, od, :], ps[:],
                                                   bc_sb["fc2_b"][:, od:od + 1],
                                                   y2[:, od, :], Alu.add, Alu.add)
                    hb = pg.tile([P, NQ], BF, tag="rot_hbf", bufs=2)
                    nc.scalar.copy(hb[:], h3[:, od, :])
                    s2 = pg.tile([P, NQ], BF, tag="rot_sq", bufs=2)
                    nc.vector.tensor_tensor(s2[:], hb[:], hb[:], Alu.mult)
                    hbf.append(hb)
                    sq.append(s2)
                out_f = pg.tile([P, DC, NQ], F32, tag="af32", bufs=2)
                _layernorm(nc, pa, pools, cfg, h3, hbf, sq,
                           bc_sb["ln3_g"], bc_sb["ln3_b"], out_f, None)
                nc.sync.dma_start(outT_d.rearrange("(c p) n -> p c n", p=P),
                                  out_f[:])

    nc.compile()
    return nc


def make_in_maps(cfg, inputs):
    B, T, S, D, H, F = (cfg[k] for k in "BTSDHF")
    NQ = T // 2
    DC, FC = D // P, F // P
    HD = D // H
    bf = ml_dtypes.bfloat16

    def col(v):  # [D'] -> [P, D'//P]
        return np.ascontiguousarray(np.asarray(v, np.float32).reshape(-1, P).T)

    def wtile(w):  # [DO, DI] -> [DO/P, P, DI/P, P] od-tiles of transposed weight
        w = np.asarray(w, np.float32)
        do, di = w.shape
        return np.ascontiguousarray(
            w.reshape(do // P, P, di // P, P).transpose(0, 3, 2, 1)).astype(bf)

    shared = {}
    sc = HD ** -0.5
    shared["sa_wqT"] = wtile(np.asarray(inputs["sa_wq"]) * sc)
    shared["ca_wqT"] = wtile(np.asarray(inputs["ca_wq"]) * sc)
    for nm in ("sa_wk", "sa_wo", "ca_wk", "ca_wo"):
        shared[nm + "T"] = wtile(inputs[nm])
    for nm in ("sa_wv", "ca_wv"):
        shared[nm + "T"] = np.ascontiguousarray(
            np.asarray(inputs[nm], np.float32).T).astype(bf)
    shared["f1T"] = wtile(inputs["fc1_w"])
    shared["f2T"] = wtile(inputs["fc2_w"])
    shared["sa_bq_c"] = col(np.asarray(inputs["sa_bq"]) * sc)
    shared["ca_bq_c"] = col(np.asarray(inputs["ca_bq"]) * sc)
    for nm in ("sa_bk", "sa_bo", "ca_bk", "ca_bo", "fc2_b", "fc1_b",
               "ln1_g", "ln1_b", "ln2_g", "ln2_b", "ln3_g", "ln3_b"):
        shared[nm + "_c"] = col(inputs[nm])
    shared["sa_bv_row"] = np.asarray(inputs["sa_bv"], np.float32)[None, :].astype(bf)
    shared["ca_bv_row"] = np.asarray(inputs["ca_bv"], np.float32)[None, :].astype(bf)

    in_maps = []
    for c in range(2 * B):
        b, half = divmod(c, 2)
        qs = slice(half * NQ, (half + 1) * NQ)
        x = np.asarray(inputs["hidden_states"][b], np.float32)  # [T, D]
        m = {}
        m.update(shared)
        m["xT"] = np.ascontiguousarray(x.T).astype(bf)
        m["xqT"] = np.ascontiguousarray(x[qs].T).astype(bf)
        m["xres"] = np.ascontiguousarray(x[qs].T)
        m["encT"] = np.ascontiguousarray(
            np.asarray(inputs["encoder_hidden_states"][b], np.float32).T).astype(bf)
        m["mskT"] = np.ascontiguousarray(
            np.asarray(inputs["attention_mask"][b, 0], np.float32)[qs].T).astype(bf)
        m["emskT"] = np.ascontiguousarray(
            np.asarray(inputs["encoder_attention_mask"][b, 0], np.float32)[qs].T).astype(bf)
        in_maps.append(m)
    return in_maps


_NC_CACHE = {}


def get_nc(cfg=None):
    cfg = cfg or default_cfg()
    key = tuple(sorted((k, str(v)) for k, v in cfg.items()))
    if key not in _NC_CACHE:
        _NC_CACHE[key] = build_nc(dict(cfg))
    return _NC_CACHE[key]


def kernel(**inputs):
    from concourse.bass_utils import run_bass_kernel_spmd

    cfg = default_cfg()
    B, T, D = cfg["B"], cfg["T"], cfg["D"]
    NQ = T // 2
    nc = get_nc(cfg)
    in_maps = make_in_maps(cfg, inputs)
    res = run_bass_kernel_spmd(nc, in_maps, list(range(2 * B))).results
    out = np.empty((B, T, D), np.float32)
    for c in range(2 * B):
        b, half = divmod(c, 2)
        out[b, half * NQ:(half + 1) * NQ, :] = res[c]["outT"].T
    return out


# revision 29
# speedup vs baseline: 1.1525x; 1.1525x over previous
"""BART decoder layer on 8 TRN2 NeuronCores.

Sharding: data-parallel over (batch, query-half): core c handles batch c//2,
query rows [half*512, half*512+512). Each core computes the full decoder layer
for its 512 query tokens; self/cross K,V are recomputed per core from the full
batch sequence (no collectives).

On-device layout is "transposed": activations live as [feature, token] so every
matmul contracts along the SBUF partition axis. Weights are pre-transposed on
the host and cast to bf16; accumulation is f32 in PSUM, residuals/LayerNorm are
f32. Softmax skips max-subtraction (scores are O(1) here); row sums come from an
extra ones-column appended to V. LayerNorm partition-axis sums use ones-matmuls
on the TensorEngine.
"""

import sys

sys.path.insert(0, "/opt/trn_rl_repo")

import ml_dtypes
import numpy as np

import concourse.bacc as bacc
import concourse.bass as bass
import concourse.mybir as mybir
import concourse.tile as tile

BF = mybir.dt.bfloat16
F32 = mybir.dt.float32
P = 128
Act = mybir.ActivationFunctionType
Alu = mybir.AluOpType


def default_cfg():
    return dict(B=4, T=1024, S=1024, D=1024, H=16, F=4096, eps=1e-5,
                gelu=Act.Gelu, self_mask=True, cross_mask=False)


def _attention(nc, pa, pools, cfg, kv_dram, L, q_sb, msk_dram, res_sb,
               wq_d, wk_d, wv_d, wo_d, bq_c, bk_c, bo_c, bv_row, h_f32, pg):
    """One multi-head attention block, fully in transposed layout.

    pa: phase-scoped SBUF pool.
    kv_dram: [D, L] bf16 dram AP (source tokens for K/V)
    q_sb:    [P, DC, NQ] bf16 sbuf (source for Q)
    msk_dram:[L, NQ] bf16 dram AP of exp(mask) factors, or None (no masking)
    res_sb:  [P, DC, NQ] f32 sbuf (residual)
    wq_d/wk_d/wo_d: [DC, P, DC, P] host-tiled bf16; wv_d: [D, D] bf16
    h_f32:   [P, DC, NQ] f32 sbuf out (attn_out + bias + residual)
    Returns (h_bf_tiles, sq_tiles) lists used by LayerNorm stats.
    """
    D, H, NQ = cfg["D"], cfg["H"], cfg["NQ"]
    HD = D // H
    DC, LC = D // P, L // P
    HPC = P // HD  # heads per 128-row chunk
    KB = min(512, L)  # K-proj column block
    VB = min(512, D)  # V-proj column block
    pw, pps, psa, ppv, psm = (pools[k] for k in
                              ("w", "ps_proj", "ps_score", "ps_pv", "small"))

    kv_sb = pa.tile([P, DC, L], BF, tag="kvsrc")
    nc.sync.dma_start(kv_sb[:], kv_dram.rearrange("(c p) n -> p c n", p=P))
    if msk_dram is not None:
        msk_sb = pa.tile([P, LC, NQ], BF, tag="msk")
        nc.sync.dma_start(msk_sb[:], msk_dram.rearrange("(c p) n -> p c n", p=P))

    # V bias broadcast row -> [P, D]
    vbias = pa.tile([P, D], BF, tag="vbias")
    nc.gpsimd.partition_broadcast(vbias[:], bv_row[:1, :])

    # K^T [d_out, k]  (ksb[p, od, k])
    ksb = pa.tile([P, DC, L], BF, tag="ksb")
    for od in range(DC):
        wk_t = pw.tile([P, DC, P], BF, tag="wod", bufs=3)
        nc.sync.dma_start(wk_t[:], wk_d[od])
        for nb in range(L // KB):
            ps = pps.tile([P, KB], F32, tag="proj")
            for c in range(DC):
                nc.tensor.matmul(ps[:], wk_t[:, c, :],
                                 kv_sb[:, c, nb * KB:(nb + 1) * KB],
                                 start=(c == 0), stop=(c == DC - 1))
            nc.scalar.activation(ksb[:, od, nb * KB:(nb + 1) * KB], ps[:],
                                 Act.Identity, bias=bk_c[:, od:od + 1])

    # V natural layout + ones column: vsb[p, t, h*65:(h+1)*65], col 64 of head = 1
    wv_sb = pw.tile([P, DC, D], BF, tag="wvfull", bufs=1)
    nc.sync.dma_start(wv_sb[:], wv_d.rearrange("(c p) n -> p c n", p=P))
    vsb = pa.tile([P, LC, H * (HD + 1)], BF, tag="vsb")
    vsb_r = vsb[:].rearrange("p t (h w) -> p t h w", w=HD + 1)
    for tt in range(LC):
        nc.vector.memset(vsb_r[:, tt, :, HD:HD + 1], 1.0)
        for nb in range(D // VB):
            ps = pps.tile([P, VB], F32, tag="proj")
            for c in range(DC):
                nc.tensor.matmul(ps[:], kv_sb[:, c, tt * P:(tt + 1) * P],
                                 wv_sb[:, c, nb * VB:(nb + 1) * VB],
                                 start=(c == 0), stop=(c == DC - 1))
            hpb = VB // HD  # heads per block
            nc.vector.tensor_tensor(
                vsb_r[:, tt, nb * hpb:(nb + 1) * hpb, :HD],
                ps[:].rearrange("p (h w) -> p h w", w=HD),
                vbias[:, nb * VB:(nb + 1) * VB].rearrange("p (h w) -> p h w", w=HD),
                Alu.add)

    # Q^T [d_out, q]
    qsb = pa.tile([P, DC, NQ], BF, tag="qsb")
    for od in range(DC):
        wq_t = pw.tile([P, DC, P], BF, tag="wod", bufs=3)
        nc.sync.dma_start(wq_t[:], wq_d[od])
        ps = pps.tile([P, NQ], F32, tag="proj")
        for c in range(DC):
            nc.tensor.matmul(ps[:], wq_t[:, c, :], q_sb[:, c, :],
                             start=(c == 0), stop=(c == DC - 1))
        nc.scalar.activation(qsb[:, od, :], ps[:], Act.Identity,
                             bias=bq_c[:, od:od + 1])

    # attention per head -> osb [d, q] bf16 (unnormalized; batch-normalized below)
    osb = pa.tile([P, DC, NQ], BF, tag="osb")
    for h in range(H):
        od, po = divmod(h, HPC)
        po *= HD
        pT = pa.tile([P, LC, NQ], BF, tag="pT", bufs=cfg.get("pt_bufs", 2))
        for kc in range(LC):
            ps_s = psa.tile([P, NQ], F32, tag="score")
            nc.tensor.matmul(ps_s[:], ksb[po:po + HD, od, kc * P:(kc + 1) * P],
                             qsb[po:po + HD, od, :], start=True, stop=True)
            if msk_dram is not None:
                if cfg.get("mask_mult", True):
                    sc = pa.tile([P, NQ], BF, tag="sc", bufs=2)
                    nc.scalar.activation(sc[:], ps_s[:], Act.Exp)
                    nc.vector.tensor_tensor(pT[:, kc, :], sc[:], msk_sb[:, kc, :],
                                            Alu.mult)
                else:
                    sc = pa.tile([P, NQ], F32, tag="sc", bufs=2)
                    nc.vector.tensor_tensor(sc[:], ps_s[:], msk_sb[:, kc, :],
                                            Alu.add)
                    nc.scalar.activation(pT[:, kc, :], sc[:], Act.Exp)
            else:
                nc.scalar.activation(pT[:, kc, :], ps_s[:], Act.Exp)
        ps_o = ppv.tile([HD + 1, NQ], F32, tag="pv")
        for kc in range(LC):
            nc.tensor.matmul(ps_o[:], vsb[:, kc, h * (HD + 1):(h + 1) * (HD + 1)],
                             pT[:, kc, :], start=(kc == 0), stop=(kc == LC - 1))
        rl = psm.tile([1, NQ], F32, tag="rl", bufs=2)
        if cfg.get("recip_fast", True):
            # custom-DVE ops misread PSUM at partition offset 64; stage the
            # denominator row to a partition-0 SBUF tile first
            rls = psm.tile([1, NQ], F32, tag="rls", bufs=2)
            nc.scalar.copy(rls[:], ps_o[HD:HD + 1, :])
            nc.vector.reciprocal_approx_fast(rl[:], rls[:])
        else:
            nc.vector.reciprocal(rl[:], ps_o[HD:HD + 1, :])
        rb = psm.tile([HD, NQ], F32, tag="rb", bufs=2)
        nc.gpsimd.partition_broadcast(rb[:], rl[:1, :])
        nc.vector.tensor_tensor(osb[po:po + HD, od, :], ps_o[:HD, :], rb[:],
                                Alu.mult)

    if cfg.get("dbg"):
        pfx = cfg["_dbg_pfx"]
        for nm, t in (("ksb", ksb), ("vsb", vsb), ("qsb", qsb), ("osb", osb)):
            d = nc.declare_dram_parameter(f"dbg_{pfx}_{nm}", list(t.shape),
                                          t.dtype, isOutput=True)
            nc.sync.dma_start(d[:], t[:])

    # out-proj + bias + residual -> h_f32 (f32); emit bf16 + square tiles for LN
    h_bf_tiles, sq_tiles = [], []
    for od in range(DC):
        wo_t = pw.tile([P, DC, P], BF, tag="wod", bufs=3)
        nc.sync.dma_start(wo_t[:], wo_d[od])
        ps = pps.tile([P, NQ], F32, tag="proj")
        for c in range(DC):
            nc.tensor.matmul(ps[:], wo_t[:, c, :], osb[:, c, :],
                             start=(c == 0), stop=(c == DC - 1))
        nc.vector.scalar_tensor_tensor(h_f32[:, od, :], ps[:], bo_c[:, od:od + 1],
                                       res_sb[:, od, :], Alu.add, Alu.add)
        hb = pg.tile([P, NQ], BF, tag="rot_hbf", bufs=2)
        nc.scalar.copy(hb[:], h_f32[:, od, :])
        sq = pg.tile([P, NQ], BF, tag="rot_sq", bufs=2)
        nc.vector.tensor_tensor(sq[:], hb[:], hb[:], Alu.mult)
        h_bf_tiles.append(hb)
        sq_tiles.append(sq)
    return h_bf_tiles, sq_tiles


def _layernorm(nc, pg, pools, cfg, h_f32, h_bf_tiles, sq_tiles, g_c, b_c,
               y_f32, y_bf):
    """y = LN(h) over the partition (feature) axis via ones-matmul stats."""
    D, NQ, eps = cfg["D"], cfg["NQ"], cfg["eps"]
    DC = D // P
    psm, pstat = pools["small"], pools["ps_score"]
    ones_bf = pools["ones_bf"]

    ps_sx = pstat.tile([1, NQ], F32, tag="score")
    for c in range(DC):
        nc.tensor.matmul(ps_sx[:], ones_bf[:], h_bf_tiles[c][:],
                         start=(c == 0), stop=(c == DC - 1))
    ps_sq = pstat.tile([1, NQ], F32, tag="score")
    for c in range(DC):
        nc.tensor.matmul(ps_sq[:], ones_bf[:], sq_tiles[c][:],
                         start=(c == 0), stop=(c == DC - 1))

    m = psm.tile([1, NQ], F32, tag="m")
    nc.vector.tensor_scalar_mul(m[:], ps_sx[:], 1.0 / D)
    ms = psm.tile([1, NQ], F32, tag="ms")
    nc.vector.tensor_scalar_mul(ms[:], ps_sq[:], 1.0 / D)
    var = psm.tile([1, NQ], F32, tag="var")
    nc.vector.tensor_tensor(var[:], m[:], m[:], Alu.mult)
    nc.vector.tensor_sub(var[:], ms[:], var[:])
    nc.vector.tensor_scalar_add(var[:], var[:], float(eps))
    sd = psm.tile([1, NQ], F32, tag="sd")
    nc.scalar.activation(sd[:], var[:], Act.Sqrt)
    rstd = psm.tile([1, NQ], F32, tag="rstd")
    nc.vector.reciprocal(rstd[:], sd[:])
    cc = psm.tile([1, NQ], F32, tag="cc")
    nc.vector.tensor_tensor(cc[:], m[:], rstd[:], Alu.mult)
    nc.vector.tensor_scalar_mul(cc[:], cc[:], -1.0)

    ab = psm.tile([P, NQ], F32, tag="ab")
    nc.gpsimd.partition_broadcast(ab[:], rstd[:1, :])
    cb = psm.tile([P, NQ], F32, tag="cb")
    nc.gpsimd.partition_broadcast(cb[:], cc[:1, :])

    for c in range(DC):
        t1 = pg.tile([P, NQ], F32, tag="rot_f32", bufs=2)
        nc.vector.tensor_tensor(t1[:], h_f32[:, c, :], ab[:], Alu.mult)
        nc.vector.tensor_tensor(t1[:], t1[:], cb[:], Alu.add)
        nc.vector.tensor_scalar(y_f32[:, c, :], t1[:], g_c[:, c:c + 1],
                                b_c[:, c:c + 1], Alu.mult, Alu.add)
        if y_bf is not None:
            nc.scalar.copy(y_bf[:, c, :], y_f32[:, c, :])


def build_nc(cfg):
    B, T, S, D, H, F = (cfg[k] for k in "BTSDHF")
    NQ = cfg["NQ"] = T // 2
    DC, TC, SC, FC = D // P, T // P, S // P, F // P
    HD = D // H

    nc = bacc.Bacc("TRN2", target_bir_lowering=False,
                   debug=cfg.get("debug", False), num_devices=2 * B)
    dp = nc.declare_dram_parameter
    xT_d = dp("xT", [D, T], BF, isOutput=False)
    xqT_d = dp("xqT", [D, NQ], BF, isOutput=False)
    xres_d = dp("xres", [D, NQ], F32, isOutput=False)
    encT_d = dp("encT", [D, S], BF, isOutput=False)
    mskT_d = dp("emT", [T, NQ], BF, isOutput=False) if cfg["self_mask"] else None
    emskT_d = dp("cemT", [S, NQ], BF, isOutput=False) if cfg["cross_mask"] else None
    w_d = {}
    for nm in ("sa_wq", "sa_wk", "sa_wo", "ca_wq", "ca_wk", "ca_wo"):
        w_d[nm] = dp(nm + "T", [DC, P, DC, P], BF, isOutput=False)
    for nm in ("sa_wv", "ca_wv"):
        w_d[nm] = dp(nm + "T", [D, D], BF, isOutput=False)
    f1_d = dp("f1T", [FC, P, DC, P], BF, isOutput=False)
    f2_d = dp("f2T", [DC, P, FC, P], BF, isOutput=False)
    bias_cols = {}
    for nm in ("sa_bq", "sa_bk", "sa_bo", "ca_bq", "ca_bk", "ca_bo",
               "fc2_b", "ln1_g", "ln1_b", "ln2_g", "ln2_b", "ln3_g", "ln3_b"):
        bias_cols[nm] = dp(nm + "_c", [P, DC], F32, isOutput=False)
    bias_cols["fc1_b"] = dp("fc1_b_c", [P, FC], F32, isOutput=False)
    bvrow_d = dp("sa_bv_row", [1, D], BF, isOutput=False)
    cvrow_d = dp("ca_bv_row", [1, D], BF, isOutput=False)
    outT_d = dp("outT", [D, NQ], F32, isOutput=True)

    with tile.TileContext(nc) as tc:
        with tc.tile_pool(name="const", bufs=1) as pc, \
             tc.tile_pool(name="glob", bufs=1) as pg, \
             tc.tile_pool(name="wpool", bufs=1) as pw, \
             tc.tile_pool(name="small", bufs=1) as psm, \
             tc.tile_pool(name="ps_proj", bufs=4, space="PSUM") as pps, \
             tc.tile_pool(name="ps_score", bufs=2, space="PSUM") as psa, \
             tc.tile_pool(name="ps_pv", bufs=2, space="PSUM") as ppv:

            # constants
            ones_bf = pc.tile([P, 1], BF)
            nc.vector.memset(ones_bf[:], 1.0)
            bc_sb = {}
            for nm, d in bias_cols.items():
                t = pc.tile(list(d.shape), F32, tag="bc_" + nm)
                nc.sync.dma_start(t[:], d[:])
                bc_sb[nm] = t
            bvrow_sb = pc.tile([1, D], BF, tag="bvrow_sa")
            nc.sync.dma_start(bvrow_sb[:], bvrow_d[:])
            cvrow_sb = pc.tile([1, D], BF, tag="bvrow_ca")
            nc.sync.dma_start(cvrow_sb[:], cvrow_d[:])

            pools = dict(w=pw, small=psm, ps_proj=pps, ps_score=psa,
                         ps_pv=ppv, ones_bf=ones_bf)

            # globals: residual-chain f32 slots and q-source bf16 slots
            xq_sb = pg.tile([P, DC, NQ], BF, tag="qsrc", bufs=2)
            nc.sync.dma_start(xq_sb[:], xqT_d.rearrange("(c p) n -> p c n", p=P))
            xres_sb = pg.tile([P, DC, NQ], F32, tag="af32", bufs=2)
            nc.sync.dma_start(xres_sb[:], xres_d.rearrange("(c p) n -> p c n", p=P))

            # ---- self attention + LN1 ----
            h1 = pg.tile([P, DC, NQ], F32, tag="af32", bufs=2)
            cfg["_dbg_pfx"] = "sa"
            with tc.tile_pool(name="attn1", bufs=1) as pa:
                hbf, sq = _attention(nc, pa, pools, cfg, xT_d, T, xq_sb, mskT_d,
                                     xres_sb, w_d["sa_wq"], w_d["sa_wk"],
                                     w_d["sa_wv"], w_d["sa_wo"],
                                     bc_sb["sa_bq"], bc_sb["sa_bk"], bc_sb["sa_bo"],
                                     bvrow_sb, h1, pg)
                y1 = pg.tile([P, DC, NQ], F32, tag="af32", bufs=2)
                y1b = pg.tile([P, DC, NQ], BF, tag="qsrc", bufs=2)
                _layernorm(nc, pg, pools, cfg, h1, hbf, sq,
                           bc_sb["ln1_g"], bc_sb["ln1_b"], y1, y1b)

            # ---- cross attention + LN2 ----
            h2 = pg.tile([P, DC, NQ], F32, tag="af32", bufs=2)
            cfg["_dbg_pfx"] = "ca"
            with tc.tile_pool(name="attn2", bufs=1) as pa:
                hbf, sq = _attention(nc, pa, pools, cfg, encT_d, S, y1b, emskT_d,
                                     y1, w_d["ca_wq"], w_d["ca_wk"],
                                     w_d["ca_wv"], w_d["ca_wo"],
                                     bc_sb["ca_bq"], bc_sb["ca_bk"], bc_sb["ca_bo"],
                                     cvrow_sb, h2, pg)
                y2 = pg.tile([P, DC, NQ], F32, tag="af32", bufs=2)
                y2b = pg.tile([P, DC, NQ], BF, tag="qsrc", bufs=2)
                _layernorm(nc, pg, pools, cfg, h2, hbf, sq,
                           bc_sb["ln2_g"], bc_sb["ln2_b"], y2, y2b)

            if cfg.get("dbg"):
                for nm, t in (("h1", h1), ("y1", y1), ("h2", h2), ("y2", y2)):
                    d = dp(f"dbg_{nm}", list(t.shape), t.dtype, isOutput=True)
                    nc.sync.dma_start(d[:], t[:])

            # ---- FFN + LN3 ----
            with tc.tile_pool(name="ffn", bufs=1) as pa:
                fsb = pa.tile([P, FC, NQ], BF, tag="fsb")
                for ft in range(FC):
                    w1 = pw.tile([P, DC, P], BF, tag="wod", bufs=3)
                    nc.sync.dma_start(w1[:], f1_d[ft])
                    ps = pps.tile([P, NQ], F32, tag="proj")
                    for c in range(DC):
                        nc.tensor.matmul(ps[:], w1[:, c, :], y2b[:, c, :],
                                         start=(c == 0), stop=(c == DC - 1))
                    nc.scalar.activation(fsb[:, ft, :], ps[:], cfg["gelu"],
                                         bias=bc_sb["fc1_b"][:, ft:ft + 1])
                h3 = pg.tile([P, DC, NQ], F32, tag="af32", bufs=2)
                hbf, sq = [], []
                for od in range(DC):
                    w2 = pa.tile([P, FC, P], BF, tag="w2", bufs=2)
                    nc.sync.dma_start(w2[:], f2_d[od])
                    ps = pps.tile([P, NQ], F32, tag="proj")
                    for fc_ in range(FC):
                        nc.tensor.matmul(ps[:], w2[:, fc_, :], fsb[:, fc_, :],
                                         start=(fc_ == 0), stop=(fc_ == FC - 1))
                    nc.vector.scalar_tensor_tensor(h3[:, od, :], ps[:],
                                                   bc_sb["fc2_b"][:, od:od + 1],
                                                   y2[:, od, :], Alu.add, Alu.add)
                    hb = pg.tile([P, NQ], BF, tag="rot_hbf", bufs=2)
                    nc.scalar.copy(hb[:], h3[:, od, :])
                    s2 = pg.tile([P, NQ], BF, tag="rot_sq", bufs=2)
                    nc.vector.tensor_tensor(s2[:], hb[:], hb[:], Alu.mult)
                    hbf.append(hb)
                    sq.append(s2)
                out_f = pg.tile([P, DC, NQ], F32, tag="af32", bufs=2)
                _layernorm(nc, pg, pools, cfg, h3, hbf, sq,
                           bc_sb["ln3_g"], bc_sb["ln3_b"], out_f, None)
                nc.sync.dma_start(outT_d.rearrange("(c p) n -> p c n", p=P),
                                  out_f[:])

    nc.compile()
    return nc


def make_in_maps(cfg, inputs):
    B, T, S, D, H, F = (cfg[k] for k in "BTSDHF")
    NQ = T // 2
    DC, FC = D // P, F // P
    HD = D // H
    bf = ml_dtypes.bfloat16

    def col(v):  # [D'] -> [P, D'//P]
        return np.ascontiguousarray(np.asarray(v, np.float32).reshape(-1, P).T)

    def wtile(w):  # [DO, DI] -> [DO/P, P, DI/P, P] od-tiles of transposed weight
        w = np.asarray(w, np.float32)
        do, di = w.shape
        return np.ascontiguousarray(
            w.reshape(do // P, P, di // P, P).transpose(0, 3, 2, 1)).astype(bf)

    shared = {}
    sc = HD ** -0.5
    shared["sa_wqT"] = wtile(np.asarray(inputs["sa_wq"]) * sc)
    shared["ca_wqT"] = wtile(np.asarray(inputs["ca_wq"]) * sc)
    for nm in ("sa_wk", "sa_wo", "ca_wk", "ca_wo"):
        shared[nm + "T"] = wtile(inputs[nm])
    for nm in ("sa_wv", "ca_wv"):
        shared[nm + "T"] = np.ascontiguousarray(
            np.asarray(inputs[nm], np.float32).T).astype(bf)
    shared["f1T"] = wtile(inputs["fc1_w"])
    shared["f2T"] = wtile(inputs["fc2_w"])
    shared["sa_bq_c"] = col(np.asarray(inputs["sa_bq"]) * sc)
    shared["ca_bq_c"] = col(np.asarray(inputs["ca_bq"]) * sc)
    for nm in ("sa_bk", "sa_bo", "ca_bk", "ca_bo", "fc2_b", "fc1_b",
               "ln1_g", "ln1_b", "ln2_g", "ln2_b", "ln3_g", "ln3_b"):
        shared[nm + "_c"] = col(inputs[nm])
    shared["sa_bv_row"] = np.asarray(inputs["sa_bv"], np.float32)[None, :].astype(bf)
    shared["ca_bv_row"] = np.asarray(inputs["ca_bv"], np.float32)[None, :].astype(bf)

    in_maps = []
    for c in range(2 * B):
        b, half = divmod(c, 2)
        qs = slice(half * NQ, (half + 1) * NQ)
        x = np.asarray(inputs["hidden_states"][b], np.float32)  # [T, D]
        m = {}
        m.update(shared)
        m["xT"] = np.ascontiguousarray(x.T).astype(bf)
        m["xqT"] = np.ascontiguousarray(x[qs].T).astype(bf)
        m["xres"] = np.ascontiguousarray(x[qs].T)
        m["encT"] = np.ascontiguousarray(
            np.asarray(inputs["encoder_hidden_states"][b], np.float32).T).astype(bf)
        if cfg.get("self_mask", True):
            m["emT"] = np.ascontiguousarray(np.exp(
                np.asarray(inputs["attention_mask"][b, 0], np.float32)[qs].T)).astype(bf)
        if cfg.get("cross_mask", False):
            m["cemT"] = np.ascontiguousarray(np.exp(
                np.asarray(inputs["encoder_attention_mask"][b, 0], np.float32)[qs].T)).astype(bf)
        in_maps.append(m)
    return in_maps


_NC_CACHE = {}


def get_nc(cfg=None):
    cfg = cfg or default_cfg()
    key = tuple(sorted((k, str(v)) for k, v in cfg.items()))
    if key not in _NC_CACHE:
        _NC_CACHE[key] = build_nc(dict(cfg))
    return _NC_CACHE[key]


def kernel(**inputs):
    from concourse.bass_utils import run_bass_kernel_spmd

    cfg = default_cfg()
    cfg["self_mask"] = bool(np.any(np.asarray(inputs["attention_mask"])))
    cfg["cross_mask"] = bool(np.any(np.asarray(inputs["encoder_attention_mask"])))
    B, T, D = cfg["B"], cfg["T"], cfg["D"]
    NQ = T // 2
    nc = get_nc(cfg)
    in_maps = make_in_maps(cfg, inputs)
    res = run_bass_kernel_spmd(nc, in_maps, list(range(2 * B))).results
    out = np.empty((B, T, D), np.float32)
    for c in range(2 * B):
        b, half = divmod(c, 2)
        out[b, half * NQ:(half + 1) * NQ, :] = res[c]["outT"].T
    return out
